# revision 3
# baseline (speedup 1.0000x reference)
"""Trainium2 Bass kernel for nn_Meta_Graph1_40114994545303 (gnn_message_passing).

Math: only the global-node row of the GCN output is returned, so
    out[s, :] = tanh( (sum_a mask[s,a] * attribute_feat[s,a,:]) @ W + b )
and x never reaches the output (adj[A, A] = 0).

Sharding: d_in split across 8 cores (256 cols each): per core ~2.06MB of
compacted live feat rows + 1MB W slice + 0.33MB mask blocks in, 1MB fp16
partial out; host epilogue sums the 8 partials + bias + tanh (the unshard).

v3 vs v2 (38.3us): PE-transpose via identity instead of xbar DMA-transpose
(was 4x1.2us serialized + PE re-throttle), pipelined by sample half
(stage-2 of samples 0-127 overlaps the feat stream of samples 128-255),
warmup matmuls to lift the HAM clock gate before real work, W mid-stream,
finer feat pieces, and per-half split output DMAs on both HWDGE queues.
"""

import numpy as np

import concourse.bacc as bacc
import concourse.mybir as mybir

B, A, D = 256, 32, 2048
NCORES = 8
DS = D // NCORES  # 256 d_in columns per core
P = 128
F32 = mybir.dt.float32
F16 = mybir.dt.float16
F8 = mybir.dt.float8e4
N_WARM = 40


def build_nc(nch: int, sched: tuple):
    """sched = tuple of (chunk, group, first, last); group g accumulates into
    PSUM window [32*(g%4):+32, 256*(g//4):+256] of the agg bank. Entries with
    g<4 (sample half 0) form a prefix; the kernel pipelines by half."""
    nmm = len(sched)
    h0_entries = [(i, e) for i, e in enumerate(sched) if e[1] < 4]
    h1_entries = [(i, e) for i, e in enumerate(sched) if e[1] >= 4]
    assert h0_entries[-1][0] + 1 == h1_entries[0][0], "halves must be contiguous"
    nm0 = len(h0_entries)

    # feat DMA piece boundaries (chunk indices)
    cuts = sorted(set(min(c, nch) for c in (0, 3, 10, 17, 22, 28, nch)))
    pieces = [(cuts[i], cuts[i + 1]) for i in range(len(cuts) - 1)]
    npieces = len(pieces)

    def piece_of(c):
        for pi, (a0, a1) in enumerate(pieces):
            if a0 <= c < a1:
                return pi
        raise AssertionError

    nc = bacc.Bacc("TRN2", target_bir_lowering=False, debug=False)

    featd = nc.dram_tensor("feat", [P, nch * DS], F16, kind="ExternalInput")
    mbdtd = nc.dram_tensor("mbdt", [P, nmm * 32], F8, kind="ExternalInput")
    wd = nc.dram_tensor("w", [P, 2 * D], F16, kind="ExternalInput")
    identd = nc.dram_tensor("ident", [P, P], F16, kind="ExternalInput")
    outd = nc.dram_tensor("out", [B, D], F16, kind="ExternalOutput")

    from contextlib import ExitStack

    with ExitStack() as ctx:
        feat_sb = ctx.enter_context(nc.sbuf_tensor([P, nch, DS], F16))
        mbdt_sb = ctx.enter_context(nc.sbuf_tensor([P, nmm, 32], F8))
        w_sb = ctx.enter_context(nc.sbuf_tensor([P, 2, D], F16))
        ident_sb = ctx.enter_context(nc.sbuf_tensor([P, P], F16))
        agg_sb = ctx.enter_context(nc.sbuf_tensor([P, 512], F16))
        aggT_sb = ctx.enter_context(nc.sbuf_tensor([P, 2, 2, P], F16))
        out_sb = ctx.enter_context(nc.sbuf_tensor([P, 2, D], F16))
        pm_agg = ctx.enter_context(nc.psum_tensor("pm_agg", [P, 512], F32))
        pt = ctx.enter_context(nc.psum_tensor("pt", [P, 1024], F16))
        pb = [
            ctx.enter_context(nc.psum_tensor(f"pb{i}", [P, 512], F32))
            for i in range(4)
        ]
        fsems = [ctx.enter_context(nc.semaphore(f"fs{g}")) for g in range(npieces)]
        msems = [ctx.enter_context(nc.semaphore(f"ms{j}")) for j in range(2)]
        wsems = [ctx.enter_context(nc.semaphore(f"ws{k}")) for k in range(2)]
        isem = ctx.enter_context(nc.semaphore("isem"))
        s1h = [ctx.enter_context(nc.semaphore(f"s1h{h}")) for h in range(2)]
        cpag = [ctx.enter_context(nc.semaphore(f"cpag{h}")) for h in range(2)]
        trh = [ctx.enter_context(nc.semaphore(f"trh{h}")) for h in range(2)]
        cpq = [ctx.enter_context(nc.semaphore(f"cpq{h}")) for h in range(2)]
        s2h = [ctx.enter_context(nc.semaphore(f"s2h{h}")) for h in range(2)]
        cpd = [ctx.enter_context(nc.semaphore(f"cpd{h}")) for h in range(2)]
        cpa2 = [ctx.enter_context(nc.semaphore(f"cpa2{h}")) for h in range(2)]
        osem = ctx.enter_context(nc.semaphore("osem"))
        block = ctx.enter_context(nc.Block(no_gpsimd_drain=True))

        def feat_dma(eng, pi):
            a0, a1 = pieces[pi]
            eng.dma_start(
                feat_sb[:, a0:a1, :],
                featd[:, a0 * DS : a1 * DS].rearrange("p (c d) -> p c d", d=DS),
            ).then_inc(fsems[pi], 16)

        @block.sync
        def _(sync):
            for pi in (0, 1, 2):
                feat_dma(sync, pi)
            sync.dma_start(w_sb[:, 0, :], wd[:, 0:D]).then_inc(wsems[0], 16)
            feat_dma(sync, 4)
            for h in range(2):
                sync.wait_ge(cpd[h], 2)
                sync.dma_start(
                    outd[128 * h : 128 * h + 128, 0:1024], out_sb[:, h, 0:1024]
                ).then_inc(osem, 16)
            sync.wait_ge(osem, 64)

        @block.scalar
        def _(scalar):
            scalar.dma_start(ident_sb[:], identd[:]).then_inc(isem, 16)
            scalar.dma_start(
                mbdt_sb[:, 0:nm0, :],
                mbdtd[:, 0 : nm0 * 32].rearrange("p (m j) -> p m j", j=32),
            ).then_inc(msems[0], 16)
            feat_dma(scalar, 3)
            scalar.dma_start(w_sb[:, 1, :], wd[:, D : 2 * D]).then_inc(wsems[1], 16)
            scalar.dma_start(
                mbdt_sb[:, nm0:nmm, :],
                mbdtd[:, nm0 * 32 : nmm * 32].rearrange("p (m j) -> p m j", j=32),
            ).then_inc(msems[1], 16)
            feat_dma(scalar, 5)
            # ACT copies banks n=2,3 of each half, then ships that half's
            # right 1024 output columns
            for h in range(2):
                for n in (2, 3):
                    scalar.wait_ge(s2h[h], n + 1)
                    nc.scalar.activation(
                        out_sb[:, h, 512 * n : 512 * n + 512],
                        pb[n][:],
                        mybir.ActivationFunctionType.Copy,
                    ).then_inc(cpa2[h], 1)
                scalar.wait_ge(cpa2[h], 2)
                scalar.dma_start(
                    outd[128 * h : 128 * h + 128, 1024:2048],
                    out_sb[:, h, 1024:2048],
                ).then_inc(osem, 16)

        @block.vector
        def _(vector):
            for h in range(2):
                vector.wait_ge(s1h[h], 1)
                nc.vector.tensor_copy(
                    agg_sb[:, 256 * h : 256 * h + 256],
                    pm_agg[:, 256 * h : 256 * h + 256],
                ).then_inc(cpag[h], 1)
                vector.wait_ge(trh[h], 2)
                for k in range(2):
                    nc.vector.tensor_copy(
                        aggT_sb[:, k, h, :],
                        pt[:, 256 * h + 128 * k : 256 * h + 128 * k + 128],
                    ).then_inc(cpq[h], 1)
                for n in (0, 1):
                    vector.wait_ge(s2h[h], n + 1)
                    nc.vector.tensor_copy(
                        out_sb[:, h, 512 * n : 512 * n + 512], pb[n][:]
                    ).then_inc(cpd[h], 1)

        @block.tensor
        def _(tensor):
            # warmup against the HAM clock gate; out_sb is scratch here
            for i in range(N_WARM):
                nc.tensor.matmul(
                    pb[0][0:32, 0:64],
                    out_sb[0:P, 0, 0:32],
                    out_sb[0:P, 0, 64:128],
                    start=True,
                    stop=True,
                    skip_group_check=True,
                )
            seen = set()

            def s1_pass(entries, h):
                tensor.wait_ge(msems[h], 16)
                last = None
                for i, (c, g, first, lastf) in entries:
                    pi = piece_of(c)
                    if pi not in seen:
                        seen.add(pi)
                        tensor.wait_ge(fsems[pi], 16)
                    last = nc.tensor.matmul(
                        pm_agg[
                            32 * (g % 4) : 32 * (g % 4) + 32,
                            256 * (g // 4) : 256 * (g // 4) + 256,
                        ],
                        mbdt_sb[:, i, :],
                        feat_sb[:, c, :],
                        start=first,
                        stop=lastf,
                        tile_position=(0, 32 * (g % 4)),
                        skip_group_check=True,
                    )
                last.then_inc(s1h[h], 1)

            def transpose_pass(h):
                tensor.wait_ge(cpag[h], 1)
                if h == 0:
                    tensor.wait_ge(isem, 16)
                for k in range(2):
                    nc.tensor.transpose(
                        pt[:, 256 * h + 128 * k : 256 * h + 128 * k + 128],
                        agg_sb[:, 256 * h + 128 * k : 256 * h + 128 * k + 128],
                        ident_sb[:],
                    ).then_inc(trh[h], 1)

            def s2_pass(h):
                tensor.wait_ge(cpq[h], 2)
                if h == 1:
                    # banks pb0-3 are reused; half 0's copies must be done
                    tensor.wait_ge(cpd[0], 2)
                    tensor.wait_ge(cpa2[0], 2)
                for k in range(2):
                    if h == 0:
                        tensor.wait_ge(wsems[k], 16)
                    for n in range(4):
                        mm = nc.tensor.matmul(
                            pb[n][:],
                            aggT_sb[:, k, h, :],
                            w_sb[:, k, 512 * n : 512 * n + 512],
                            start=(k == 0),
                            stop=(k == 1),
                            skip_group_check=True,
                        )
                        if k == 1:
                            mm.then_inc(s2h[h], 1)

            s1_pass(h0_entries, 0)
            transpose_pass(0)
            s2_pass(0)
            s1_pass(h1_entries, 1)
            transpose_pass(1)
            s2_pass(1)

    nc.compile()
    return nc


def _host_prep(inputs: dict):
    feat = np.asarray(inputs["attribute_feat"], dtype=np.float32)
    label = np.asarray(inputs["attribute_label"])
    mask = label > 0  # [B, A]

    s_idx, a_idx = np.nonzero(mask)
    n_live = len(s_idx)
    nch = -(-n_live // P)
    n_pad = nch * P
    row_s = np.full(n_pad, -1, np.int64)
    row_s[:n_live] = s_idx

    feat_all = np.zeros((n_pad, D), np.float16)
    feat_all[:n_live] = feat[s_idx, a_idx].astype(np.float16)

    sched = []
    blocks = []
    g_seen = set()
    for c in range(nch):
        rs = row_s[c * P : (c + 1) * P]
        gs = sorted({int(s) // 32 for s in rs if s >= 0})
        for g in gs:
            blk = np.zeros((P, 32), np.float16)
            sel = (rs >= 32 * g) & (rs < 32 * (g + 1))
            blk[np.nonzero(sel)[0], rs[sel] - 32 * g] = 1.0
            sched.append([c, g, g not in g_seen, False])
            g_seen.add(g)
            blocks.append(blk)
    g_last = {}
    for i, (c, g, f, _) in enumerate(sched):
        g_last[g] = i
    for g, i in g_last.items():
        sched[i][3] = True
    sched = tuple(tuple(e) for e in sched)
    mbdt16 = np.concatenate(blocks, axis=1)  # [128, nmm*32]
    import ml_dtypes
    f8 = getattr(ml_dtypes, "float8_e4m3fn", None) or ml_dtypes.float8_e4m3
    mbdt = mbdt16.astype(np.float32).astype(f8)  # 0/1 are exact in e4m3
    ident = np.eye(P, dtype=np.float16)

    in_maps = []
    for c in range(NCORES):
        fslice = feat_all[:, c * DS : (c + 1) * DS]
        featp = np.ascontiguousarray(
            fslice.reshape(nch, P, DS).transpose(1, 0, 2).reshape(P, nch * DS)
        )
        wslice = np.asarray(inputs["W"], dtype=np.float32)[
            c * DS : (c + 1) * DS, :
        ].astype(np.float16)
        wp = np.ascontiguousarray(
            wslice.reshape(2, P, D).transpose(1, 0, 2).reshape(P, 2 * D)
        )
        in_maps.append({"feat": featp, "mbdt": mbdt, "w": wp, "ident": ident})
    return in_maps, nch, sched


_NC_CACHE: dict = {}


def run(inputs: dict, trace: bool = False):
    from concourse.bass_utils import run_bass_kernel_spmd

    in_maps, nch, sched = _host_prep(inputs)
    key = (nch, sched)
    if key not in _NC_CACHE:
        _NC_CACHE[key] = build_nc(nch, sched)
    nc = _NC_CACHE[key]
    last_err = None
    for _ in range(3):
        try:
            res = run_bass_kernel_spmd(nc, in_maps, list(range(NCORES)), trace=trace)
            break
        except Exception as e:  # noqa: BLE001 - device transients
            last_err = e
    else:
        raise last_err
    z = np.zeros((B, D), np.float32)
    for c in range(NCORES):
        z += res.results[c]["out"].astype(np.float32)
    z += np.asarray(inputs["b"], dtype=np.float32)[None, :]
    return np.tanh(z), res


def kernel(**inputs) -> np.ndarray:
    out, _ = run(inputs)
    return out
